# revision 26
# baseline (speedup 1.0000x reference)
"""Trainium2 Bass kernel for MultiHeadDoublyStochasticSelfAttention.

Problem: b=8, n=1024, f=768, h=12, d=64; 3-step Sinkhorn (eps=1, row/col/row)
on softmax-free exp scores, then attn @ v and output projection.

Sharding: one batch element per NeuronCore (8 cores). Weights replicated.

Math (per head), single exp pass, all in exp domain:
  E^T[j,i] = exp(k_j . q_i)            (d^-0.5 folded into Wq on host)
  r_i   = sum_j E^T[j,i]               (PE ones-matvec, j on partitions)
  c~_j  = sum_i E^T[j,i] * (1/r_i)     (DVE scalar_tensor_tensor vs the
                                        broadcast 1/r row, fused accum)
  vs    = [v | N] * (1/c~_j)           (GpSimd tensor_scalar)
  A[d,i] = sum_j vs[j,d] E^T[j,i]      (PE attn@v, raw E^T)
  row 64 of A = N sum_j E^T[j,i]/c~_j  -> out^T[d,i] = A[d,i] * 1/row64[i]
(Per-i factors cancel in the A/row64 ratio, so attn@v uses the raw E^T.)
Then out^T = Wo @ concat_heads(out^T_head) + bo, host transposes back.

Engine budget drove the design: DVE RECIPROCAL is ~3.1us/instruction
regardless of size, so all three reciprocal consumers (1/r row, 1/c~
column scales, 1/(N T) gamma row) are bounced into one [128,24] "station"
tile and inverted with a single reciprocal per pipeline window. The
per-head chain is spread over 5 windows (S^T+exp+rowsum / colsums /
attn@v / gamma) so every cross-engine hop has a window of slack.
QKV projections run as fp8 DoubleRow matmuls (2 contraction rows per
partition, host-packed) at 2x PE throughput.
"""

import sys

if "/opt/trn_rl_repo" not in sys.path:
    sys.path.insert(0, "/opt/trn_rl_repo")

from contextlib import ExitStack

import numpy as np

import concourse.bass as bass
import concourse.mybir as mybir
import concourse.tile as tile

B, N, F, H, D = 8, 1024, 768, 12, 64
PC = F // 128        # 6 f-chunks of 128
TC = N // 128        # 8 token chunks of 128
KB = F // 256        # 3 DoubleRow contraction blocks (256 rows each)
NH = N // 2
WSCALE = 64.0        # host premultiplier on fp8 weights (descaled in copies)
F32 = mybir.dt.float32
BF16 = mybir.dt.bfloat16
FP8 = mybir.dt.float8e4
EXP = mybir.ActivationFunctionType.Exp
IDENT = mybir.ActivationFunctionType.Identity
MUL = mybir.AluOpType.mult
DR = mybir.MatmulPerfMode.DoubleRow


def _split_multi_waits(bir_bytes):
    """This container's walrus accepts at most ONE sync wait per instruction
    ("Too many sync wait commands"). Tile's semaphore pass attaches several.
    Rewrite the BIR: spill all but the last wait of each instruction onto
    same-engine NoOps placed directly before it (engines are in-order, so
    semantics are identical)."""
    import json

    d = json.loads(bir_bytes)
    uid = 0
    for fn in d["functions"]:
        for blk in fn["blocks"]:
            out = []
            for ins in blk["instructions"]:
                si = ins.get("sync_info")
                waits = (si or {}).get("on_wait") or []
                if len(waits) > 1:
                    for w in waits[:-1]:
                        uid += 1
                        out.append({
                            "debug": ins.get("debug", 0),
                            "engine": ins["engine"],
                            "ins": [], "outs": [],
                            "name": f"{ins['name']}-w{uid}",
                            "opcode": "NoOp",
                            "sync_info": {"on_update": [], "on_wait": [w]},
                            "text_hint": "split_wait",
                        })
                    si["on_wait"] = [waits[-1]]
                out.append(ins)
            blk["instructions"] = out
    return json.dumps(d).encode()


def build():
    nc = bass.Bass()
    x_dr = nc.declare_dram_parameter("x_dr", [KB, 128, 2, N], FP8, isOutput=False)
    wq_dr = nc.declare_dram_parameter("wq_dr", [KB, 128, 2, F], FP8, isOutput=False)
    wk_dr = nc.declare_dram_parameter("wk_dr", [KB, 128, 2, F], FP8, isOutput=False)
    wv_dr = nc.declare_dram_parameter("wv_dr", [KB, 128, 2, F], FP8, isOutput=False)
    woT = nc.declare_dram_parameter("woT", [F, F], BF16, isOutput=False)
    bo = nc.declare_dram_parameter("bo", [F], F32, isOutput=False)
    outT = nc.declare_dram_parameter("outT", [F, N], F32, isOutput=True)
    rrow_d = nc.dram_tensor("rrow_d", [H, N], F32)    # raw row sums r
    a1_d = nc.dram_tensor("a1_d", [H, N], F32)        # 1/r rows
    growt_d = nc.dram_tensor("growt_d", [H, N], F32)  # raw N*T gamma rows
    ginv_d = nc.dram_tensor("ginv_d", [H, N], F32)    # 1/(N*T) rows

    with tile.TileContext(nc) as tc, ExitStack() as ctx:
        perm = ctx.enter_context(tc.tile_pool(name="perm", bufs=1))
        qt = [perm.tile([128, N], BF16, name=f"qt{i}", tag=f"qt{i}") for i in range(PC)]
        kt = [perm.tile([128, N], BF16, name=f"kt{i}", tag=f"kt{i}") for i in range(PC)]
        # v augmented with a column of N (for the gamma row) per head
        vg = [perm.tile([128, H * (D + 1)], BF16, name=f"vg{i}", tag=f"vg{i}")
              for i in range(TC)]
        ofT = [perm.tile([128, N], BF16, name=f"ofT{i}", tag=f"ofT{i}")
               for i in range(PC)]
        wo_sb = [perm.tile([128, F], BF16, name=f"wo{i}", tag=f"wo{i}")
                 for i in range(PC)]
        bo_sb = perm.tile([128, PC], F32, name="bo_sb", tag="bo_sb")
        ones_sb = perm.tile([128, 1], BF16, name="ones_sb", tag="ones_sb")
        nc.vector.memset(ones_sb, 1.0)
        nc.sync.dma_start(out=bo_sb, in_=bo[:].rearrange("(c p) -> p c", p=128))
        for i in range(PC):
            nc.sync.dma_start(out=wo_sb[i], in_=woT[i * 128:(i + 1) * 128, :])
        for t in range(TC):
            # fill with N; the v-projection copies below overwrite the value
            # columns, leaving each head's 65th column = N (gamma-row trick)
            nc.vector.memset(vg[t], float(N))

        # ------- Phase A: q^T, k^T, v projections (fp8 DoubleRow) -------
        with tc.tile_pool(name="pxt", bufs=1) as pxt, \
             tc.tile_pool(name="pw", bufs=3 * KB) as pw, \
             tc.tile_pool(name="ppsa", bufs=3, space="PSUM") as ppsa:
            xt = [pxt.tile([128, 2, N], FP8, name=f"xt{i}", tag=f"xt{i}")
                  for i in range(KB)]
            for i in range(KB):
                nc.sync.dma_start(out=xt[i], in_=x_dr[i, :, :, :])

            w_all = {}
            for wname, wdram in (("q", wq_dr), ("k", wk_dr), ("v", wv_dr)):
                ws = []
                for kc in range(KB):
                    w = pw.tile([128, 2, F], FP8, name=f"w{wname}{kc}", tag="w")
                    nc.sync.dma_start(out=w, in_=wdram[kc, :, :, :])
                    ws.append(w)
                w_all[wname] = ws

            descale = 1.0 / WSCALE
            for wname, dst in (("q", qt), ("k", kt)):
                w_sb = w_all[wname]
                for mc in range(PC):
                    ps = ppsa.tile([128, N], F32, name="ps_a", tag="psa")
                    for kc in range(KB):
                        for hf in range(2):
                            nc.tensor.matmul(
                                ps[:, hf * NH:(hf + 1) * NH],
                                (w_sb[kc][:, :, mc * 128:(mc + 1) * 128]),
                                (xt[kc][:, :, hf * NH:(hf + 1) * NH]),
                                start=(kc == 0), stop=(kc == KB - 1),
                                perf_mode=DR, skip_group_check=True,
                            )
                    nc.scalar.mul(dst[mc], ps, descale)

            wv_sb = w_all["v"]
            for t in range(TC):
                ps = ppsa.tile([128, N], F32, name="ps_v", tag="psa")
                for kc in range(KB):
                    for hf, fw in ((0, NH), (1, F - NH)):
                        nc.tensor.matmul(
                            ps[:, hf * NH:hf * NH + fw],
                            (xt[kc][:, :, t * 128:(t + 1) * 128]),
                            (wv_sb[kc][:, :, hf * NH:hf * NH + fw]),
                            start=(kc == 0), stop=(kc == KB - 1),
                            perf_mode=DR, skip_group_check=True,
                        )
                src = ps[:, :F].rearrange("p (h e) -> p h e", e=D)
                dst3 = vg[t].rearrange("p (h e) -> p h e", e=D + 1)
                nc.vector.tensor_scalar_mul(dst3[:, :, 0:D], src, descale)

        # ---------------- Phase B: per-head sinkhorn attention ----------------
        # 5-deep software pipeline, one head per window:
        #   window h   : S^T matmuls + exp -> E^T, row-sum matvec -> r
        #   window h+1 : (r bounced to station; station reciprocal -> 1/r)
        #   window h+2 : col sums c~ (DVE STT vs broadcast 1/r row)
        #   window h+3 : vs scaling (GpSimd) + attn@v matmuls; gamma row out
        #   window h+4 : station reciprocal -> 1/(N T); broadcast
        #   window h+5 : gamma multiply -> ofT   (slot 3)
        # Station layout [128, 24]: cols 0-7 = r of head t-1 ((c p) bounced),
        # cols 8-15 = c~ accums of head t-2, cols 16-23 = gamma row of head
        # t-4. ONE reciprocal instruction per window serves all three.
        pe0t = ctx.enter_context(tc.tile_pool(name="pe0t", bufs=34))
        pscr = ctx.enter_context(tc.tile_pool(name="pscr", bufs=2))
        pa1 = ctx.enter_context(tc.tile_pool(name="pa1", bufs=2))
        psml = ctx.enter_context(tc.tile_pool(name="psml", bufs=2))
        pvs = ctx.enter_context(tc.tile_pool(name="pvs", bufs=4))
        pps_s = ctx.enter_context(tc.tile_pool(name="pps_s", bufs=2, space="PSUM"))
        pps_av = ctx.enter_context(tc.tile_pool(name="pps_av", bufs=2, space="PSUM"))
        # pass-1 windows of consecutive heads never overlap: one buffer
        pps_r = ctx.enter_context(tc.tile_pool(name="pps_r", bufs=1, space="PSUM"))

        LAG = 2
        NS = 8 + LAG

        def qk(h):
            hc, off = divmod(h, 2)
            off *= D
            return qt[hc][off:off + D, :], kt[hc][off:off + D, :]

        state = {"e": {}, "av": {}, "rbc": {}, "gb": {}}
        for t in range(H + 5):
            hA = t if t < H else None                 # pass 1
            hB = t - 2 if 0 <= t - 2 < H else None    # col sums (STT)
            hC = t - 3 if 0 <= t - 3 < H else None    # vs + attn@v
            hG = t - 5 if 0 <= t - 5 < H else None    # gamma multiply

            station = state.pop("station_next", None)
            need_station = hA is not None or hB is not None or hC is not None
            if station is None and need_station:
                station = psml.tile([128, 24], F32, name="station", tag="station")
                nc.vector.memset(station, 1.0)
            if need_station:
                station_next = psml.tile([128, 24], F32, name="station",
                                         tag="station")
                nc.vector.memset(station_next, 1.0)
                state["station_next"] = station_next

            if hA is not None:
                qA, kA = qk(hA)
                r1 = pps_r.tile([1, N], F32, name="r_ps", tag="r")
                eA = [None] * TC
                state["e"][hA] = eA
            if hB is not None:
                rbcB = state["rbc"].pop(hB)
                eB = state["e"][hB]
            if hC is not None:
                avC = pps_av.tile([D + 1, N], F32, name="av_ps", tag="av")
                state["av"][hC] = avC
                eC = state["e"][hC]
                invC = state["inv_prev"]   # cols 8-15 = 1/c~ of hC
                vsC = [None] * TC

            for s in range(NS):
                # gamma multiply of head hG (gb broadcast landed last window)
                if s == 3 and hG is not None:
                    avG = state["av"].pop(hG)
                    gbG = state["gb"].pop(hG)
                    hcz, offz = divmod(hG, 2)
                    offz *= D
                    nc.vector.tensor_mul(
                        ofT[hcz][offz:offz + D, :], avG[0:D, :], gbG
                    )

                # pass 1: S^T scores + exp (PSUM bank per 512-wide half)
                if hA is not None and s < TC:
                    e_sb = pe0t.tile([128, N], BF16, name="e_sb", tag="E")
                    eA[s] = e_sb
                    for ih in range(2):
                        ps = pps_s.tile([128, NH], F32, name="ps_s", tag="ps")
                        nc.tensor.matmul(
                            ps,
                            kA[:, s * 128:(s + 1) * 128],
                            qA[:, ih * NH:(ih + 1) * NH],
                            start=True, stop=True,
                        )
                        nc.scalar.activation(
                            e_sb[:, ih * NH:(ih + 1) * NH], ps, EXP
                        )

                # pass 1: raw row-sum matvec r_i = sum_j E^T[j,i]
                jc = s - LAG
                if hA is not None and 0 <= jc < TC:
                    for ih in range(2):
                        nc.tensor.matmul(
                            r1[:, ih * NH:(ih + 1) * NH],
                            ones_sb,
                            (eA[jc][:, ih * NH:(ih + 1) * NH]),
                            start=(jc == 0), stop=(jc == TC - 1),
                            skip_group_check=True,
                        )
                    if jc == TC - 1:
                        # bounce r into next window's station, (c p) layout
                        rrow = pa1.tile([1, N], F32, name="rrow", tag="rrow")
                        nc.scalar.copy(rrow, r1)
                        nc.sync.dma_start(out=rrow_d[hA:hA + 1, :], in_=rrow)
                        nc.sync.dma_start(
                            out=state["station_next"][:, 0:8],
                            in_=rrow_d[hA:hA + 1, :].rearrange(
                                "o (c p) -> (o p) c", p=128),
                        )

                # col sums of head hB: c~_j = sum_i E^T[j,i] * (1/r_i)
                jc = s - LAG
                if hB is not None and 0 <= jc < TC:
                    scr = pscr.tile([128, N], BF16, name="scr", tag="scr")
                    with nc.allow_low_precision(reason="bf16 scratch"):
                        nc.vector.scalar_tensor_tensor(
                            scr, eB[jc], 1.0, rbcB, MUL, MUL,
                            accum_out=station[:, 8 + jc:9 + jc],
                        )

                # attn@v of head hC: vs = [v|N]*(1/c~) then accumulate matmuls
                if hC is not None and s < TC:
                    vs = pvs.tile([128, D + 1], BF16, name="vs", tag="vs")
                    vsC[s] = vs
                    nc.gpsimd.tensor_scalar_mul(
                        vs, vg[s][:, hC * (D + 1):(hC + 1) * (D + 1)],
                        invC[:, 8 + s:9 + s],
                    )
                jc = s - LAG
                if hC is not None and 0 <= jc < TC:
                    for ih in range(2):
                        nc.tensor.matmul(
                            avC[:, ih * NH:(ih + 1) * NH],
                            vsC[jc],
                            (eC[jc][:, ih * NH:(ih + 1) * NH]),
                            start=(jc == 0), stop=(jc == TC - 1),
                            skip_group_check=True,
                        )

            # ---- window tail ----
            if hC is not None:
                # bounce the raw gamma row (N*T) into the next station
                growrow = pa1.tile([1, N], F32, name="growrow", tag="growrow")
                nc.scalar.copy(growrow, avC[D:D + 1, :])
                nc.sync.dma_start(out=growt_d[hC:hC + 1, :], in_=growrow)
                nc.sync.dma_start(
                    out=state["station_next"][:, 16:24],
                    in_=growt_d[hC:hC + 1, :].rearrange(
                        "o (c p) -> (o p) c", p=128),
                )
                del state["e"][hC]

            if station is not None:
                # ONE reciprocal: 1/r (head t-1), 1/c~ (t-2), 1/(N T) (t-4)
                inv = psml.tile([128, 24], F32, name="inv", tag="inv")
                nc.vector.reciprocal(inv, station)
                state["inv_prev"] = inv

                hR = t - 1            # head whose 1/r row is in cols 0-7
                if 0 <= hR < H:
                    nc.sync.dma_start(
                        out=a1_d[hR:hR + 1, :].rearrange(
                            "o (c p) -> (o p) c", p=128),
                        in_=inv[:, 0:8],
                    )
                    rbc = pa1.tile([128, N], F32, name="rbc", tag="rbc")
                    asrc = a1_d[hR:hR + 1, :]
                    nc.sync.dma_start(
                        out=rbc,
                        in_=bass.AP(tensor=asrc.tensor, offset=asrc.offset,
                                    ap=[[0, 128]] + list(asrc.ap[1:])),
                    )
                    state["rbc"][hR] = rbc

                hGn = t - 4           # head whose 1/(N T) row is in cols 16-23
                if 0 <= hGn < H:
                    nc.sync.dma_start(
                        out=ginv_d[hGn:hGn + 1, :].rearrange(
                            "o (c p) -> (o p) c", p=128),
                        in_=inv[:, 16:24],
                    )
                    gb = psml.tile([D, N], F32, name="gb_sb", tag="gb")
                    gsrc = ginv_d[hGn:hGn + 1, :]
                    nc.sync.dma_start(
                        out=gb,
                        in_=bass.AP(tensor=gsrc.tensor, offset=gsrc.offset,
                                    ap=[[0, D]] + list(gsrc.ap[1:])),
                    )
                    state["gb"][hGn] = gb

        # ---------------- Phase C: output projection + bias ----------------
        for mc in range(PC):
            o_sb = pscr.tile([128, N], F32, name="o_sb", tag="osb")
            for hf in range(2):
                ps = pps_s.tile([128, NH], F32, name="ps_o", tag="ps")
                for kc in range(PC):
                    nc.tensor.matmul(
                        ps,
                        (wo_sb[kc][:, mc * 128:(mc + 1) * 128]),
                        (ofT[kc][:, hf * NH:(hf + 1) * NH]),
                        start=(kc == 0), stop=(kc == PC - 1),
                    )
                nc.scalar.activation(
                    o_sb[:, hf * NH:(hf + 1) * NH], ps, IDENT,
                    bias=bo_sb[:, mc:mc + 1],
                )
            nc.sync.dma_start(out=outT[mc * 128:(mc + 1) * 128, :], in_=o_sb)

    orig_to_json = nc.to_json_bytes
    nc.to_json_bytes = lambda: _split_multi_waits(orig_to_json())
    return nc


_NC = None


def _get_nc():
    global _NC
    if _NC is None:
        _NC = build()
    return _NC


def make_in_maps(x, Wq, Wk, Wv, Wo, bo):
    import ml_dtypes

    bf16 = ml_dtypes.bfloat16
    f8 = mybir.dt.np(FP8)
    scale = np.float32(D ** -0.5)

    def pack_w(w, pre):
        wt = (np.asarray(w, dtype=np.float32) * pre).T  # [f_in, f_out]
        return np.ascontiguousarray(wt.reshape(KB, 128, 2, F).astype(f8))

    wq_p = pack_w(Wq, scale * WSCALE)
    wk_p = pack_w(Wk, WSCALE)
    wv_p = pack_w(Wv, WSCALE)
    wo_t = np.ascontiguousarray(np.asarray(Wo).T.astype(bf16))
    bo_c = np.ascontiguousarray(np.asarray(bo).astype(np.float32))
    maps = []
    for c in range(B):
        xt = np.asarray(x[c]).T.astype(np.float32)  # [F, N]
        x_p = np.ascontiguousarray(xt.reshape(KB, 128, 2, N).astype(f8))
        maps.append({
            "x_dr": x_p,
            "wq_dr": wq_p, "wk_dr": wk_p, "wv_dr": wv_p,
            "woT": wo_t, "bo": bo_c,
        })
    return maps


def kernel(x, Wq, Wk, Wv, Wo, bo):
    from concourse.bass_utils import run_bass_kernel_spmd

    x = np.asarray(x)
    nc = _get_nc()
    in_maps = make_in_maps(x, Wq, Wk, Wv, Wo, bo)
    res = run_bass_kernel_spmd(nc, in_maps, core_ids=list(range(B)))
    out = np.stack([res.results[c]["outT"].T.astype(np.float32) for c in range(B)],
                   axis=0)
    return out
